# revision 8
# baseline (speedup 1.0000x reference)
"""MHA kernel for TRN2: x[8,512,32,32], 8 heads, S=1024, C=512.

Sharding: data-parallel over batch N=8 -> one batch item per NeuronCore.
Per-core layout (all transpose-free):
  qkT[e,s]  = w_qkvT[:, :1024].T @ x      (e on partitions; q tiles 0-3, k tiles 4-7)
  v[s,e]    = x.T @ w_qkvT[:, 1024:]      (s on partitions, natural layout)
  scoresT   = kT_h.T @ qT_h               (k_s on partitions; K=64 -> head pair packed
                                           at PE rows 0-63 / 64-127, 2x via row tiling)
  P         = exp(scoresT/8)              per psq tile: cols [0,FA) exact exp on the
                                           scalar engine; cols [FA,1024) Schraudolph
                                           bit-trick exp on DVE (affine -> int16 round,
                                           bitcast to bf16)
  oT_aug    = [v_h | 1].T @ P             (M=65; row 64 = softmax denominator r)
  oT        = oT_aug[:64] * (1/r)         (DVE evac+recip, gpsimd broadcast+multiply)
  yT[o,s]   = w_outT.T @ oT               (accumulates in the psq banks freed after the
                                           last QK; + b_out added host-side)

Single software-pipelined pass: PV trails QK by LAG slots within each head
pair; v-proj and the next pair's q/k-proj groups fill the tensor gaps.
"""

import numpy as np
import ml_dtypes

import concourse.bacc as bacc
import concourse.mybir as mybir
import concourse.tile as tile
from concourse.bass_utils import run_bass_kernel_spmd

P = 128
S = 1024          # sequence = 32*32
C = 512           # channels
NH = 8            # heads
HD = 64           # head dim
CT = C // P       # 4 c-tiles
MT = S // P       # 8 s-tiles
BF = mybir.dt.bfloat16
I16 = mybir.dt.int16
F32 = mybir.dt.float32

LAG = 2           # PV trails QK by this many mt slots


def is_dve_tile(pair, nt, mt):
    """Score tiles routed to DVE (Schraudolph exp); rest go to the scalar
    engine (exact exp). Whole-tile assignment avoids cross-engine WAW
    serialization on the pt tile. Pair 0 is tensor-bound (v-proj fillers),
    so it keeps everything on ACT."""
    if pair == 0:
        return False
    return (nt * MT + mt) % 3 == 1

SCALE = 1.0 / np.sqrt(HD)
SCH_A = float(SCALE * np.log2(np.e) * 128.0)       # Schraudolph slope
SCH_B = float(127.0 * 128.0 - 7.0 + 0.5)           # bias - magic + round-half

_cache = {}


def build_program(dbg=False):
    nc = bacc.Bacc("TRN2", target_bir_lowering=False, debug=False, num_devices=8)
    x_d = nc.dram_tensor("x", [C, S], BF, kind="ExternalInput").ap()
    wq_d = nc.dram_tensor("wq", [C, 3 * C], BF, kind="ExternalInput").ap()
    wo_d = nc.dram_tensor("wo", [C, C], BF, kind="ExternalInput").ap()
    y_d = nc.dram_tensor("y", [C, S], BF, kind="ExternalOutput").ap()
    dbg_d = {}
    if dbg:
        for nm, shp, dt in (("qk0", [P, S], BF), ("qk4", [P, S], BF),
                            ("v0", [P, NH * (HD + 1)], BF),
                            ("pt00", [P, 1024], BF), ("pt30", [P, 1024], BF),
                            ("oun0", [HD, 512], F32), ("rinv0", [1, 512], F32),
                            ("oT0", [P, S], BF), ("oT3", [P, S], BF)):
            dbg_d[nm] = nc.dram_tensor("dbg_" + nm, shp, dt,
                                       kind="ExternalOutput").ap()

    with tile.TileContext(nc) as tc:
        with (
            tc.tile_pool(name="const", bufs=1) as cpool,
            tc.tile_pool(name="qk", bufs=1) as qkpool,
            tc.tile_pool(name="vp", bufs=1) as vpool,
            tc.tile_pool(name="pp", bufs=6) as ppool,
            tc.tile_pool(name="ot", bufs=1) as opool,
            tc.tile_pool(name="yp", bufs=1) as ypool,
            tc.tile_pool(name="oun", bufs=12) as ounpool,
            tc.tile_pool(name="rin", bufs=12) as rinpool,
            tc.tile_pool(name="bcp", bufs=12) as bcpool,
            tc.tile_pool(name="psq", bufs=2, space="PSUM") as psq_pool,
            tc.tile_pool(name="pso", bufs=2, space="PSUM") as pso_pool,
            tc.tile_pool(name="fil", bufs=2, space="PSUM") as fil_pool,
        ):
            # ---- input DMAs (batched; earliest-needed slices first) ----
            x_sb = cpool.tile([P, CT * S], BF, name="x", tag="x")
            x3 = x_sb[:].rearrange("p (ct s) -> p ct s", ct=CT)
            w_sb = cpool.tile([P, CT * 3 * C], BF, name="w", tag="w")
            w3 = w_sb[:].rearrange("p (ct e) -> p ct e", ct=CT)
            wo_sb = cpool.tile([P, CT * C], BF, name="wo", tag="wo")
            wo3 = wo_sb[:].rearrange("p (ct o) -> p ct o", ct=CT)

            xs = x_d.rearrange("(ct p) s -> p ct s", p=P)
            ws = wq_d.rearrange("(ct p) e -> p ct e", p=P)
            wos = wo_d.rearrange("(ct p) o -> p ct o", p=P)
            nc.sync.dma_start(x3[:, 0:2, :], xs[:, 0:2, :])
            nc.sync.dma_start(x3[:, 2:4, :], xs[:, 2:4, :])
            nc.sync.dma_start(w3[:, :, 0:P], ws[:, :, 0:P])                  # q et=0
            nc.sync.dma_start(w3[:, :, 4 * P:5 * P], ws[:, :, 4 * P:5 * P])  # k et=4
            nc.sync.dma_start(w3[:, :, P:4 * P], ws[:, :, P:4 * P])
            nc.sync.dma_start(w3[:, :, 5 * P:12 * P], ws[:, :, 5 * P:12 * P])
            nc.sync.dma_start(wo3[:], wos[:])

            def x_ap(ct, lo, hi):
                return x_sb[:, ct * S + lo:ct * S + hi]

            def w_ap(ct, lo, hi):
                return w_sb[:, ct * 3 * C + lo:ct * 3 * C + hi]

            # ---- projection groups (through the single fill bank) ----
            qk_sb = [qkpool.tile([P, S], BF, name=f"qk{et}", tag=f"qk{et}")
                     for et in range(8)]
            v_sb = [vpool.tile([P, NH * (HD + 1)], BF, name=f"v{mt}", tag=f"v{mt}")
                    for mt in range(MT)]
            for mt in range(MT):
                nc.gpsimd.memset(v_sb[mt][:], 1.0)

            def emit_qk_group(et, nt, on_act):
                ps = fil_pool.tile([P, 512], F32, name="qp", tag="fil")
                for ct in range(CT):
                    nc.tensor.matmul(
                        ps[:],
                        w_ap(ct, et * P, (et + 1) * P),
                        x_ap(ct, nt * 512, (nt + 1) * 512),
                        start=(ct == 0), stop=(ct == CT - 1),
                    )
                dst = qk_sb[et][:, nt * 512:(nt + 1) * 512]
                if on_act:
                    nc.scalar.copy(dst, ps[:])
                else:
                    nc.vector.tensor_copy(dst, ps[:])

            def emit_v_group(mt):
                ps = fil_pool.tile([P, 512], F32, name="vg", tag="fil")
                for ct in range(CT):
                    nc.tensor.matmul(
                        ps[:],
                        x_ap(ct, mt * P, (mt + 1) * P),
                        w_ap(ct, 2 * C, 3 * C),
                        start=(ct == 0), stop=(ct == CT - 1),
                    )
                dst = v_sb[mt][:].rearrange("p (h e) -> p h e", e=HD + 1)[:, :, 0:HD]
                nc.vector.tensor_copy(dst, ps[:].rearrange("p (h e) -> p h e", e=HD))

            # block A: q/k tiles pair 0 needs (evac on ACT: lowest latency)
            for et, nt in ((0, 0), (4, 0), (0, 1), (4, 1)):
                emit_qk_group(et, nt, on_act=True)

            # ---- attention ----
            oT_sb = [opool.tile([P, S], BF, name=f"o{ct}", tag=f"o{ct}")
                     for ct in range(CT)]

            def emit_norm(pso_t, pair, hh, nt):
                # custom-DVE reciprocal needs a partition-0-based SBUF input:
                # copy the denominator row out first (as the baseline did).
                o_un = ounpool.tile([HD, 512], F32, name="oun", tag="oun")
                nc.scalar.copy(o_un[:], pso_t[0:HD, :])
                rrow = rinpool.tile([1, 512], F32, name="rrow", tag="rrow")
                nc.vector.tensor_copy(rrow[0:1, :], pso_t[HD:HD + 1, :])
                rinv = rinpool.tile([1, 512], F32, name="rinv", tag="rinv")
                nc.vector.reciprocal_approx_fast(rinv[0:1, :], rrow[0:1, :])
                bc = bcpool.tile([HD, 512], F32, name="bc", tag="bc")
                nc.gpsimd.partition_broadcast(bc[:], rinv[0:1, :], channels=HD)
                nc.gpsimd.tensor_mul(
                    oT_sb[pair][hh * HD:(hh + 1) * HD, nt * 512:(nt + 1) * 512],
                    o_un[:], bc[:],
                )
                if dbg and pair == 0 and hh == 0 and nt == 0:
                    nc.sync.dma_start(dbg_d["oun0"][:, :], o_un[:])
                    nc.sync.dma_start(dbg_d["rinv0"][:, :], rinv[0:1, :])

            for pair in range(NH // 2):
                fill = []
                if pair == 0:
                    fill = [("v", mt, 0) for mt in range(MT)]
                if pair < 3:
                    fill += [("qk", pair + 1, 0), ("qk", pair + 5, 0),
                             ("qk", pair + 1, 1), ("qk", pair + 5, 1)]
                fi = 0

                for nt in range(2):
                    pso_t = [pso_pool.tile([P, 512], F32, name=f"pv{hh}", tag="pso")
                             for hh in range(2)]
                    pts = {}
                    for mt in range(MT):
                        # QK for (mt, nt): both heads concurrently via row tiling
                        psq = psq_pool.tile([P, 1024], F32, name="psq", tag="psq")
                        for hh in range(2):
                            nc.tensor.matmul(
                                psq[:, hh * 512:(hh + 1) * 512],
                                qk_sb[4 + pair][hh * HD:(hh + 1) * HD,
                                                mt * P:(mt + 1) * P],
                                qk_sb[pair][hh * HD:(hh + 1) * HD,
                                            nt * 512:(nt + 1) * 512],
                                start=True, stop=True,
                            )
                        # exp: whole tile on one engine (exact on ACT, or
                        # Schraudolph bit-trick on DVE)
                        pt = ppool.tile([P, 1024], BF, name="pt", tag="pt")
                        if is_dve_tile(pair, nt, mt):
                            nc.vector.tensor_scalar(
                                pt[:].bitcast(I16), psq[:],
                                SCH_A, SCH_B,
                                mybir.AluOpType.mult, mybir.AluOpType.add,
                            )
                        else:
                            nc.scalar.activation(
                                pt[:], psq[:],
                                mybir.ActivationFunctionType.Exp,
                                scale=float(SCALE),
                            )
                        pts[mt] = pt
                        if dbg and nt == 0 and mt == 0 and pair in (0, 3):
                            nc.sync.dma_start(dbg_d[f"pt{pair}0"][:, :], pt[:])
                        # PV trailing by LAG slots
                        if mt >= LAG:
                            tgt = mt - LAG
                            for hh in range(2):
                                nc.tensor.matmul(
                                    pso_t[hh][0:HD + 1, :],
                                    v_sb[tgt][:, (2 * pair + hh) * (HD + 1):
                                              (2 * pair + hh + 1) * (HD + 1)],
                                    pts[tgt][:, hh * 512:(hh + 1) * 512],
                                    start=(tgt == 0), stop=(tgt == MT - 1),
                                )
                        # one filler group per slot until exhausted
                        if fi < len(fill):
                            g = fill[fi]
                            fi += 1
                            if g[0] == "v":
                                emit_v_group(g[1])
                            else:
                                emit_qk_group(g[1], g[2], on_act=False)
                    for tgt in range(MT - LAG, MT):
                        for hh in range(2):
                            nc.tensor.matmul(
                                pso_t[hh][0:HD + 1, :],
                                v_sb[tgt][:, (2 * pair + hh) * (HD + 1):
                                          (2 * pair + hh + 1) * (HD + 1)],
                                pts[tgt][:, hh * 512:(hh + 1) * 512],
                                start=(tgt == 0), stop=(tgt == MT - 1),
                            )
                    for hh in range(2):
                        emit_norm(pso_t[hh][:], pair, hh, nt)

            if dbg:
                nc.sync.dma_start(dbg_d["qk0"][:, :], qk_sb[0][:])
                nc.sync.dma_start(dbg_d["qk4"][:, :], qk_sb[4][:])
                nc.sync.dma_start(dbg_d["v0"][:, :], v_sb[0][:])
                nc.sync.dma_start(dbg_d["oT0"][:, :], oT_sb[0][:])
                nc.sync.dma_start(dbg_d["oT3"][:, :], oT_sb[3][:])

            # ---- output projection: reuses freed psq banks ----
            y_sb = ypool.tile([P, CT * S], BF, name="ysb", tag="ysb")
            ysrc = y_sb[:].rearrange("p (ot s) -> p ot s", ot=CT)
            ydst = y_d.rearrange("(ot p) s -> ot p s", p=P)
            for st in range(2):
                for half in range(2):
                    yp = psq_pool.tile([P, 1024], F32, name="yp", tag="psq")
                    for ct in range(CT):
                        for g in range(2):
                            ot = half * 2 + g
                            nc.tensor.matmul(
                                yp[:, g * 512:(g + 1) * 512],
                                wo_sb[:, ct * C + ot * P:ct * C + (ot + 1) * P],
                                oT_sb[ct][:, st * 512:(st + 1) * 512],
                                start=(ct == 0), stop=(ct == CT - 1),
                            )
                    for g in range(2):
                        ot = half * 2 + g
                        nc.scalar.copy(
                            y_sb[:, ot * S + st * 512:ot * S + (st + 1) * 512],
                            yp[:, g * 512:(g + 1) * 512],
                        )
                for g in range(4):
                    nc.sync.dma_start(
                        y_d[g * P:(g + 1) * P, st * 512:(st + 1) * 512],
                        y_sb[:, g * S + st * 512:g * S + (st + 1) * 512],
                    )

    nc.compile()
    return nc


def get_program(dbg=False):
    key = ("nc", dbg)
    if key not in _cache:
        _cache[key] = build_program(dbg)
    return _cache[key]


def kernel(x, w_qkv, w_out, b_out, _trace=False, _tmpdir=None, _dbg=False):
    x = np.asarray(x, dtype=np.float32)
    w_qkv = np.asarray(w_qkv, dtype=np.float32)
    w_out = np.asarray(w_out, dtype=np.float32)
    b_out = np.asarray(b_out, dtype=np.float32)
    N = x.shape[0]

    xb = x.reshape(N, C, S).astype(ml_dtypes.bfloat16)
    wqT = np.ascontiguousarray(w_qkv.T).astype(ml_dtypes.bfloat16)
    woT = np.ascontiguousarray(w_out.T).astype(ml_dtypes.bfloat16)

    nc = get_program(_dbg)
    in_maps = [
        {"x": np.ascontiguousarray(xb[n]), "wq": wqT, "wo": woT}
        for n in range(N)
    ]
    res = run_bass_kernel_spmd(
        nc, in_maps, core_ids=list(range(N)), trace=_trace, tmpdir=_tmpdir
    )
    y = np.stack([res.results[n]["y"].astype(np.float32) for n in range(N)])
    y = y.reshape(N, C, 32, 32)
    y = y + b_out[None, :, None, None]
    if _dbg:
        return y, res.results[0]
    if _trace:
        return y, res
    return y


# revision 9
# speedup vs baseline: 2.3179x; 2.3179x over previous
"""MHA kernel for TRN2: x[8,512,32,32], 8 heads, S=1024, C=512.

Sharding: data-parallel over batch N=8 -> one batch item per NeuronCore.
Per-core layout (all transpose-free):
  qkT[e,s]  = w_qkvT[:, :1024].T @ x      (e on partitions; q tiles 0-3, k tiles 4-7)
  v[s,e]    = x.T @ w_qkvT[:, 1024:]      (s on partitions, natural layout)
  scoresT   = kT_h.T @ qT_h               (k_s on partitions; K=64 -> head pair packed
                                           at PE rows 0-63 / 64-127, 2x via row tiling)
  P         = exp(scoresT/8)              per psq tile: cols [0,FA) exact exp on the
                                           scalar engine; cols [FA,1024) Schraudolph
                                           bit-trick exp on DVE (affine -> int16 round,
                                           bitcast to bf16)
  oT_aug    = [v_h | 1].T @ P             (M=65; row 64 = softmax denominator r)
  oT        = oT_aug[:64] * (1/r)         (DVE evac+recip, gpsimd broadcast+multiply)
  yT[o,s]   = w_outT.T @ oT               (accumulates in the psq banks freed after the
                                           last QK; + b_out added host-side)

Single software-pipelined pass: PV trails QK by LAG slots within each head
pair; v-proj and the next pair's q/k-proj groups fill the tensor gaps.
"""

import numpy as np
import ml_dtypes

import concourse.bacc as bacc
import concourse.mybir as mybir
import concourse.tile as tile
from concourse.bass_utils import run_bass_kernel_spmd

P = 128
S = 1024          # sequence = 32*32
C = 512           # channels
NH = 8            # heads
HD = 64           # head dim
CT = C // P       # 4 c-tiles
MT = S // P       # 8 s-tiles
BF = mybir.dt.bfloat16
I16 = mybir.dt.int16
F32 = mybir.dt.float32

LAG = 2           # PV trails QK by this many mt slots


def is_dve_tile(pair, nt, mt):
    """Score tiles routed to DVE (Schraudolph exp); rest go to the scalar
    engine (exact exp). Whole-tile assignment avoids cross-engine WAW
    serialization on the pt tile. Pair 0 is tensor-bound (v-proj fillers),
    so it keeps everything on ACT."""
    if pair == 0:
        return False
    return (nt * MT + mt) % 3 == 1

SCALE = 1.0 / np.sqrt(HD)
SCH_A = float(SCALE * np.log2(np.e) * 128.0)       # Schraudolph slope
SCH_B = float(127.0 * 128.0 - 7.0 + 0.5)           # bias - magic + round-half

_cache = {}


def build_program(dbg=False):
    nc = bacc.Bacc("TRN2", target_bir_lowering=False, debug=False, num_devices=8)
    x_d = nc.dram_tensor("x", [C, S], BF, kind="ExternalInput").ap()
    wq_d = nc.dram_tensor("wq", [C, 3 * C], BF, kind="ExternalInput").ap()
    wo_d = nc.dram_tensor("wo", [C, C], BF, kind="ExternalInput").ap()
    y_d = nc.dram_tensor("y", [C, S], BF, kind="ExternalOutput").ap()
    dbg_d = {}
    if dbg:
        for nm, shp, dt in (("qk0", [P, S], BF), ("qk4", [P, S], BF),
                            ("v0", [P, NH * (HD + 1)], BF),
                            ("pt00", [P, 1024], BF), ("pt30", [P, 1024], BF),
                            ("oun0", [HD, 512], F32), ("rinv0", [1, 512], F32),
                            ("oT0", [P, S], BF), ("oT3", [P, S], BF)):
            dbg_d[nm] = nc.dram_tensor("dbg_" + nm, shp, dt,
                                       kind="ExternalOutput").ap()

    with tile.TileContext(nc) as tc:
        with (
            tc.tile_pool(name="const", bufs=1) as cpool,
            tc.tile_pool(name="qk", bufs=1) as qkpool,
            tc.tile_pool(name="vp", bufs=1) as vpool,
            tc.tile_pool(name="pp", bufs=6) as ppool,
            tc.tile_pool(name="ot", bufs=1) as opool,
            tc.tile_pool(name="yp", bufs=1) as ypool,
            tc.tile_pool(name="oun", bufs=12) as ounpool,
            tc.tile_pool(name="rin", bufs=12) as rinpool,
            tc.tile_pool(name="bcp", bufs=12) as bcpool,
            tc.tile_pool(name="psq", bufs=2, space="PSUM") as psq_pool,
            tc.tile_pool(name="pso", bufs=2, space="PSUM") as pso_pool,
            tc.tile_pool(name="fil", bufs=2, space="PSUM") as fil_pool,
        ):
            # ---- input DMAs (batched; earliest-needed slices first) ----
            x_sb = cpool.tile([P, CT * S], BF, name="x", tag="x")
            x3 = x_sb[:].rearrange("p (ct s) -> p ct s", ct=CT)
            w_sb = cpool.tile([P, CT * 3 * C], BF, name="w", tag="w")
            w3 = w_sb[:].rearrange("p (ct e) -> p ct e", ct=CT)
            wo_sb = cpool.tile([P, CT * C], BF, name="wo", tag="wo")
            wo3 = wo_sb[:].rearrange("p (ct o) -> p ct o", ct=CT)

            xs = x_d.rearrange("(ct p) s -> p ct s", p=P)
            ws = wq_d.rearrange("(ct p) e -> p ct e", p=P)
            wos = wo_d.rearrange("(ct p) o -> p ct o", p=P)
            nc.sync.dma_start(x3[:, 0:2, :], xs[:, 0:2, :])
            nc.sync.dma_start(x3[:, 2:4, :], xs[:, 2:4, :])
            nc.sync.dma_start(w3[:, :, 0:P], ws[:, :, 0:P])                  # q et=0
            nc.sync.dma_start(w3[:, :, 4 * P:5 * P], ws[:, :, 4 * P:5 * P])  # k et=4
            nc.sync.dma_start(w3[:, :, P:4 * P], ws[:, :, P:4 * P])
            nc.sync.dma_start(w3[:, :, 5 * P:12 * P], ws[:, :, 5 * P:12 * P])
            nc.sync.dma_start(wo3[:], wos[:])

            def x_ap(ct, lo, hi):
                return x_sb[:, ct * S + lo:ct * S + hi]

            def w_ap(ct, lo, hi):
                return w_sb[:, ct * 3 * C + lo:ct * 3 * C + hi]

            # ---- projection groups (through the single fill bank) ----
            qk_sb = [qkpool.tile([P, S], BF, name=f"qk{et}", tag=f"qk{et}")
                     for et in range(8)]
            v_sb = [vpool.tile([P, NH * (HD + 1)], BF, name=f"v{mt}", tag=f"v{mt}")
                    for mt in range(MT)]
            for mt in range(MT):
                nc.vector.memset(v_sb[mt][:], 1.0)

            def emit_qk_group(et, nt, on_act):
                ps = fil_pool.tile([P, 512], F32, name="qp", tag="fil")
                for ct in range(CT):
                    nc.tensor.matmul(
                        ps[:],
                        w_ap(ct, et * P, (et + 1) * P),
                        x_ap(ct, nt * 512, (nt + 1) * 512),
                        start=(ct == 0), stop=(ct == CT - 1),
                    )
                dst = qk_sb[et][:, nt * 512:(nt + 1) * 512]
                if on_act:
                    nc.scalar.copy(dst, ps[:])
                else:
                    nc.vector.tensor_copy(dst, ps[:])

            def emit_v_group(mt):
                ps = fil_pool.tile([P, 512], F32, name="vg", tag="fil")
                for ct in range(CT):
                    nc.tensor.matmul(
                        ps[:],
                        x_ap(ct, mt * P, (mt + 1) * P),
                        w_ap(ct, 2 * C, 3 * C),
                        start=(ct == 0), stop=(ct == CT - 1),
                    )
                dst = v_sb[mt][:].rearrange("p (h e) -> p h e", e=HD + 1)[:, :, 0:HD]
                nc.vector.tensor_copy(dst, ps[:].rearrange("p (h e) -> p h e", e=HD))

            # block A: q/k tiles pair 0 needs (evac on ACT: lowest latency)
            for et, nt in ((0, 0), (4, 0), (0, 1), (4, 1)):
                emit_qk_group(et, nt, on_act=True)

            # ---- attention ----
            oT_sb = [opool.tile([P, S], BF, name=f"o{ct}", tag=f"o{ct}")
                     for ct in range(CT)]

            def emit_norm(pso_t, pair, hh, nt):
                # custom-DVE reciprocal needs a partition-0-based SBUF input:
                # copy the denominator row out first (as the baseline did).
                o_un = ounpool.tile([HD, 512], F32, name="oun", tag="oun")
                nc.scalar.copy(o_un[:], pso_t[0:HD, :])
                rrow = rinpool.tile([1, 512], F32, name="rrow", tag="rrow")
                nc.vector.tensor_copy(rrow[0:1, :], pso_t[HD:HD + 1, :])
                rinv = rinpool.tile([1, 512], F32, name="rinv", tag="rinv")
                nc.vector.reciprocal_approx_fast(rinv[0:1, :], rrow[0:1, :])
                bc = bcpool.tile([HD, 512], F32, name="bc", tag="bc")
                nc.gpsimd.partition_broadcast(bc[:], rinv[0:1, :], channels=HD)
                # NB: keep gpsimd on a single op type (partition_broadcast) —
                # mixing op libraries forces a ~7us Q7 library swap per op.
                nc.vector.tensor_mul(
                    oT_sb[pair][hh * HD:(hh + 1) * HD, nt * 512:(nt + 1) * 512],
                    o_un[:], bc[:],
                )
                if dbg and pair == 0 and hh == 0 and nt == 0:
                    nc.sync.dma_start(dbg_d["oun0"][:, :], o_un[:])
                    nc.sync.dma_start(dbg_d["rinv0"][:, :], rinv[0:1, :])

            for pair in range(NH // 2):
                fill = []
                if pair == 0:
                    fill = [("v", mt, 0) for mt in range(MT)]
                if pair < 3:
                    fill += [("qk", pair + 1, 0), ("qk", pair + 5, 0),
                             ("qk", pair + 1, 1), ("qk", pair + 5, 1)]
                fi = 0

                for nt in range(2):
                    pso_t = [pso_pool.tile([P, 512], F32, name=f"pv{hh}", tag="pso")
                             for hh in range(2)]
                    pts = {}
                    for mt in range(MT):
                        # QK for (mt, nt): both heads concurrently via row tiling
                        psq = psq_pool.tile([P, 1024], F32, name="psq", tag="psq")
                        for hh in range(2):
                            nc.tensor.matmul(
                                psq[:, hh * 512:(hh + 1) * 512],
                                qk_sb[4 + pair][hh * HD:(hh + 1) * HD,
                                                mt * P:(mt + 1) * P],
                                qk_sb[pair][hh * HD:(hh + 1) * HD,
                                            nt * 512:(nt + 1) * 512],
                                start=True, stop=True,
                            )
                        # exp: whole tile on one engine (exact on ACT, or
                        # Schraudolph bit-trick on DVE)
                        pt = ppool.tile([P, 1024], BF, name="pt", tag="pt")
                        if is_dve_tile(pair, nt, mt):
                            nc.vector.tensor_scalar(
                                pt[:].bitcast(I16), psq[:],
                                SCH_A, SCH_B,
                                mybir.AluOpType.mult, mybir.AluOpType.add,
                            )
                        else:
                            nc.scalar.activation(
                                pt[:], psq[:],
                                mybir.ActivationFunctionType.Exp,
                                scale=float(SCALE),
                            )
                        pts[mt] = pt
                        if dbg and nt == 0 and mt == 0 and pair in (0, 3):
                            nc.sync.dma_start(dbg_d[f"pt{pair}0"][:, :], pt[:])
                        # PV trailing by LAG slots
                        if mt >= LAG:
                            tgt = mt - LAG
                            for hh in range(2):
                                nc.tensor.matmul(
                                    pso_t[hh][0:HD + 1, :],
                                    v_sb[tgt][:, (2 * pair + hh) * (HD + 1):
                                              (2 * pair + hh + 1) * (HD + 1)],
                                    pts[tgt][:, hh * 512:(hh + 1) * 512],
                                    start=(tgt == 0), stop=(tgt == MT - 1),
                                )
                        # one filler group per slot until exhausted
                        if fi < len(fill):
                            g = fill[fi]
                            fi += 1
                            if g[0] == "v":
                                emit_v_group(g[1])
                            else:
                                emit_qk_group(g[1], g[2], on_act=False)
                    for tgt in range(MT - LAG, MT):
                        for hh in range(2):
                            nc.tensor.matmul(
                                pso_t[hh][0:HD + 1, :],
                                v_sb[tgt][:, (2 * pair + hh) * (HD + 1):
                                          (2 * pair + hh + 1) * (HD + 1)],
                                pts[tgt][:, hh * 512:(hh + 1) * 512],
                                start=(tgt == 0), stop=(tgt == MT - 1),
                            )
                    for hh in range(2):
                        emit_norm(pso_t[hh][:], pair, hh, nt)

            if dbg:
                nc.sync.dma_start(dbg_d["qk0"][:, :], qk_sb[0][:])
                nc.sync.dma_start(dbg_d["qk4"][:, :], qk_sb[4][:])
                nc.sync.dma_start(dbg_d["v0"][:, :], v_sb[0][:])
                nc.sync.dma_start(dbg_d["oT0"][:, :], oT_sb[0][:])
                nc.sync.dma_start(dbg_d["oT3"][:, :], oT_sb[3][:])

            # ---- output projection: reuses freed psq banks ----
            y_sb = ypool.tile([P, CT * S], BF, name="ysb", tag="ysb")
            ysrc = y_sb[:].rearrange("p (ot s) -> p ot s", ot=CT)
            ydst = y_d.rearrange("(ot p) s -> ot p s", p=P)
            for st in range(2):
                for half in range(2):
                    yp = psq_pool.tile([P, 1024], F32, name="yp", tag="psq")
                    for ct in range(CT):
                        for g in range(2):
                            ot = half * 2 + g
                            nc.tensor.matmul(
                                yp[:, g * 512:(g + 1) * 512],
                                wo_sb[:, ct * C + ot * P:ct * C + (ot + 1) * P],
                                oT_sb[ct][:, st * 512:(st + 1) * 512],
                                start=(ct == 0), stop=(ct == CT - 1),
                            )
                    for g in range(2):
                        ot = half * 2 + g
                        nc.scalar.copy(
                            y_sb[:, ot * S + st * 512:ot * S + (st + 1) * 512],
                            yp[:, g * 512:(g + 1) * 512],
                        )
                for g in range(4):
                    nc.sync.dma_start(
                        y_d[g * P:(g + 1) * P, st * 512:(st + 1) * 512],
                        y_sb[:, g * S + st * 512:g * S + (st + 1) * 512],
                    )

    nc.compile()
    return nc


def get_program(dbg=False):
    key = ("nc", dbg)
    if key not in _cache:
        _cache[key] = build_program(dbg)
    return _cache[key]


def kernel(x, w_qkv, w_out, b_out, _trace=False, _tmpdir=None, _dbg=False):
    x = np.asarray(x, dtype=np.float32)
    w_qkv = np.asarray(w_qkv, dtype=np.float32)
    w_out = np.asarray(w_out, dtype=np.float32)
    b_out = np.asarray(b_out, dtype=np.float32)
    N = x.shape[0]

    xb = x.reshape(N, C, S).astype(ml_dtypes.bfloat16)
    wqT = np.ascontiguousarray(w_qkv.T).astype(ml_dtypes.bfloat16)
    woT = np.ascontiguousarray(w_out.T).astype(ml_dtypes.bfloat16)

    nc = get_program(_dbg)
    in_maps = [
        {"x": np.ascontiguousarray(xb[n]), "wq": wqT, "wo": woT}
        for n in range(N)
    ]
    res = run_bass_kernel_spmd(
        nc, in_maps, core_ids=list(range(N)), trace=_trace, tmpdir=_tmpdir
    )
    y = np.stack([res.results[n]["y"].astype(np.float32) for n in range(N)])
    y = y.reshape(N, C, 32, 32)
    y = y + b_out[None, :, None, None]
    if _dbg:
        return y, res.results[0]
    if _trace:
        return y, res
    return y
